# revision 11
# baseline (speedup 1.0000x reference)
"""Adaptive embedding (Transformer-XL wt103) on 8 trn2 NeuronCores.

Strategy: token-parallel across the 8 cores (2048 tokens each, no
collectives). Host sorts each core's tokens by id so that every
(bucket, 32k-row-block) becomes a contiguous segment; the device then
runs, per segment, a transposing dma_gather of the bf16 embedding rows
([d, tokens] layout, ready as the matmul stationary operand) followed by
PSUM-accumulated matmuls against the resident projection table, and
streams the projected rows out with contiguous DMAs. The host undoes the
sort permutation on the way back (unshard).

Tables are converted to bf16 host-side (rel err ~4e-3 against the f32
reference, well under the 2e-2 gate); projections are pre-transposed and
pre-scaled by sqrt(d_proj). The d=64/d=16 tables are zero-padded to 128
columns so every gather row is a multiple of 256 bytes and every matmul
runs with K=128.
"""

import os
import sys
import types

for _p in (
    "/root/.axon_site",
    "/root/.axon_site/_ro/trn_rl_repo",
    "/root/.axon_site/_ro/pypackages",
    "/opt/trn_rl_repo",
):
    if _p not in sys.path:
        sys.path.append(_p)

import numpy as np
import ml_dtypes

# antenv.axon_hooks shim: lets BASS_TRACE=1 profile runs work under axon.
try:
    import antenv.axon_hooks  # noqa: F401
except ImportError:
    _hooks = types.ModuleType("antenv.axon_hooks")
    _hooks._hook = None
    _hooks.set_axon_ntff_profile_hook = lambda h: setattr(_hooks, "_hook", h)
    _hooks.get_axon_ntff_profile_hook = lambda: _hooks._hook
    import antenv

    antenv.axon_hooks = _hooks
    sys.modules["antenv.axon_hooks"] = _hooks
    try:
        from trn_agent_boot.trn_boot import _ntff_profile_via_ctypes

        _h = _ntff_profile_via_ctypes("/opt/axon/libaxon_pjrt.so")
        if _h is not None:
            _hooks.set_axon_ntff_profile_hook(_h)
    except Exception:
        pass

import concourse.bacc as bacc
import concourse.mybir as mybir
import concourse.tile as tile
from concourse.bass_utils import run_bass_kernel_spmd

N_TOKEN = 267735
D_PROJ = 1024
CUTS = [0, 20000, 40000, 200000, N_TOKEN]
D_EMBS = [1024, 256, 64, 16]
D_PAD = [1024, 256, 128, 128]  # gathered row widths (>=128, %128)
EMB_SCALE = float(D_PROJ) ** 0.5
NCORES = 8
BLK = 32768  # int16 gather index range per segment
BF16 = ml_dtypes.bfloat16

# proj chunk bases within the packed [128, 12, 1024] projection tile
PROJ_CHUNK_BASE = [0, 8, 10, 11]

LAST_RESULT = None  # BassKernelResults of the most recent run (for test.py)


def _segments():
    """(bucket, local_row_base, id_lo, id_hi) for each gather segment."""
    segs = []
    for b in range(4):
        lo, hi = CUTS[b], CUTS[b + 1]
        for k in range(0, hi - lo, BLK):
            segs.append((b, k, lo + k, min(lo + k + BLK, hi)))
    return segs


def _build_graph(seg_plan, c_total, s_pad, rows):
    """seg_plan: list of (bucket, row_base, n_rows, n_pad, n_live, idx_colbase,
    slot_base), ordered smallest-gather-first (compute order)."""
    nc = bacc.Bacc(None, target_bir_lowering=False, debug=False, num_swdge_queues=4)
    dt = mybir.dt

    emb_par = [
        nc.declare_dram_parameter(f"embt{i}", [rows[i], D_PAD[i]], dt.bfloat16, False)
        for i in range(4)
    ]
    projs_par = nc.declare_dram_parameter("projs", [128, 12, 1024], dt.bfloat16, False)
    idx_par = nc.declare_dram_parameter("idxs", [128, c_total], dt.int16, False)
    # partition-major output: slot s lives at [s % 128, s // 128, :]
    out_par = nc.declare_dram_parameter(
        "out", [128, s_pad // 128, D_PROJ], dt.float32, True
    )

    # bucket order of first use in compute (for proj load ordering)
    border = []
    for (b, *_rest) in seg_plan:
        if b not in border:
            border.append(b)

    with tile.TileContext(nc) as tc:
        with (
            tc.tile_pool(name="const", bufs=1) as cpool,
            tc.tile_pool(name="et", bufs=1) as epool,
            tc.tile_pool(name="outs", bufs=4) as opool,
            tc.tile_pool(name="ps", bufs=8, space="PSUM") as ppool,
        ):
            # idx first, via gpsimd's own SWDGE ring so the gathers' wait
            # doesn't alias with the big projection loads' completion lane
            idx_sb = cpool.tile([128, c_total], dt.int16, tag="idx")
            nc.gpsimd.dma_start(idx_sb[:], idx_par[:])

            # per-bucket projection tiles, loaded in compute order, also on
            # SWDGE q0 behind idx (HWDGE transfers would starve the SWDGE
            # ring: a 39KB idx DMA was observed to complete only after a 3MB
            # HWDGE projection stream fully drained)
            proj_sb = [None] * 4
            for b in border:
                pcb, kc = PROJ_CHUNK_BASE[b], D_PAD[b] // 128
                pt = cpool.tile([128, kc, 1024], dt.bfloat16, tag=f"proj{b}")
                nc.gpsimd.dma_start(pt[:], projs_par[:, pcb : pcb + kc, :])
                proj_sb[b] = pt

            # Issue all gathers up front; Tile pipelines them against compute.
            etiles = []
            for g, (b, row_base, n_rows, n_pad, n_live, cb, slot) in enumerate(
                seg_plan
            ):
                kc = D_PAD[b] // 128
                et = epool.tile([128, kc, n_pad], dt.bfloat16, tag=f"et{g}")
                nc.gpsimd.dma_gather(
                    out_ap=et[:],
                    in_ap=emb_par[b][row_base : row_base + n_rows, :],
                    idxs_ap=idx_sb[:, cb : cb + n_pad // 16],
                    num_idxs=n_pad,
                    num_idxs_reg=n_pad,
                    elem_size=D_PAD[b],
                    transpose=True,
                    queue_num=1 + g % 3,
                )
                etiles.append(et)

            n_out_tiles = 0
            n_out_dmas = 0
            for g, (b, row_base, n_rows, n_pad, n_live, cb, slot) in enumerate(
                seg_plan
            ):
                kc = D_PAD[b] // 128
                et = etiles[g]
                n_tiles = n_pad // 128
                # batch up to 4 token-tiles per output DMA (contiguous in the
                # partition-major out layout)
                for tb in range(0, n_tiles, 4):
                    gsz = min(4, n_tiles - tb)
                    out_sb = opool.tile(
                        [128, 4, D_PROJ], dt.float32, tag="osb", name="osb"
                    )
                    for ti in range(gsz):
                        tt = tb + ti
                        ps = [
                            ppool.tile([128, 512], dt.float32, tag="ps", name="ps0"),
                            ppool.tile([128, 512], dt.float32, tag="ps", name="ps1"),
                        ]
                        # K-chunk outer: both N-halves reuse the same stationary
                        for c in range(kc):
                            lhsT = et[:, c, tt * 128 : (tt + 1) * 128]
                            for nh in range(2):
                                nc.tensor.matmul(
                                    ps[nh][:],
                                    lhsT,
                                    proj_sb[b][:, c, nh * 512 : (nh + 1) * 512],
                                    start=(c == 0),
                                    stop=(c == kc - 1),
                                )
                        for nh in range(2):
                            dst = out_sb[:, ti, nh * 512 : (nh + 1) * 512]
                            if (n_out_tiles + nh) % 2 == 0:
                                nc.vector.tensor_copy(dst, ps[nh][:])
                            else:
                                nc.scalar.copy(dst, ps[nh][:])
                        n_out_tiles += 1
                    t0 = slot // 128 + tb
                    eng = nc.sync if n_out_dmas % 2 == 0 else nc.scalar
                    eng.dma_start(
                        out_par[:, t0 : t0 + gsz, :], out_sb[:, :gsz, :]
                    )
                    n_out_dmas += 1

    nc.compile()
    return nc


def kernel(inp, emb0, emb1, emb2, emb3, proj0, proj1, proj2, proj3):
    global LAST_RESULT
    ids = np.asarray(inp).reshape(-1).astype(np.int64)
    n_tok = ids.shape[0]
    assert n_tok % NCORES == 0

    embs = [np.asarray(e) for e in (emb0, emb1, emb2, emb3)]
    projs = [np.asarray(p) for p in (proj0, proj1, proj2, proj3)]

    # --- stage tables (bf16, small ones zero-padded to 128 cols) ---
    embs_b = []
    for b in range(4):
        e = embs[b].astype(BF16)
        if D_PAD[b] != D_EMBS[b]:
            e = np.concatenate(
                [e, np.zeros((e.shape[0], D_PAD[b] - D_EMBS[b]), BF16)], axis=1
            )
        embs_b.append(np.ascontiguousarray(e))
    rows = [e.shape[0] for e in embs_b]

    # packed projections: projT rows, scaled, padded, rearranged to [128,12,1024]
    pt = np.zeros((1536, D_PROJ), np.float32)
    r0 = 0
    for b in range(4):
        ptb = projs[b].T * EMB_SCALE  # [d_b, 1024]
        pt[r0 : r0 + D_EMBS[b]] = ptb
        r0 += D_PAD[b]
    projs_host = np.ascontiguousarray(
        pt.reshape(12, 128, D_PROJ).transpose(1, 0, 2).astype(BF16)
    )

    # --- sort + segment + deal to cores ---
    order = np.argsort(ids, kind="stable")
    sids = ids[order]

    # collect non-empty segments, then order smallest-gather-first so the
    # TensorEngine can start on quickly-landing data
    raw = []
    for b, row_base, id_lo, id_hi in _segments():
        g_lo = np.searchsorted(sids, id_lo, "left")
        g_hi = np.searchsorted(sids, id_hi, "left")
        if g_hi == g_lo:
            continue
        raw.append((b, row_base, id_lo, id_hi, g_lo, g_hi))
    raw.sort(key=lambda r: (-(-((r[5] - r[4] + NCORES - 1) // NCORES) // 128) * 128)
             * D_PAD[r[0]])

    seg_plan = []  # (bucket, row_base, n_rows, n_pad, n_live, idx_colbase, slot)
    core_idx = [[] for _ in range(NCORES)]  # per-core int16 idx arrays per seg
    unshard = []  # (slot_base, [global token positions per core])
    cb = 0
    slot = 0
    for b, row_base, id_lo, id_hi, g_lo, g_hi in raw:
        toks = order[g_lo:g_hi]
        locs = (sids[g_lo:g_hi] - id_lo).astype(np.int16)
        counts = [len(locs[c::NCORES]) for c in range(NCORES)]
        n_live = max(counts)
        n_pad = -(-n_live // 128) * 128
        per_core_toks = []
        for c in range(NCORES):
            li = locs[c::NCORES]
            pad = np.zeros(n_pad, np.int16)
            pad[: len(li)] = li
            core_idx[c].append(pad)
            per_core_toks.append(toks[c::NCORES])
        seg_plan.append((b, row_base, id_hi - id_lo, n_pad, n_live, cb, slot))
        unshard.append((slot, per_core_toks))
        cb += n_pad // 16
        slot += n_pad
    c_total = cb
    s_pad = slot

    # --- per-core wrapped idx tensors [128, c_total] int16 ---
    in_maps = []
    for c in range(NCORES):
        flat = np.concatenate(core_idx[c])
        w = flat.reshape(-1, 16).T  # [16, c_total]
        idx_host = np.ascontiguousarray(np.tile(w, (8, 1)))
        in_maps.append(
            {
                "embt0": embs_b[0],
                "embt1": embs_b[1],
                "embt2": embs_b[2],
                "embt3": embs_b[3],
                "projs": projs_host,
                "idxs": idx_host,
            }
        )

    nc = _build_graph(seg_plan, c_total, s_pad, rows)
    res = run_bass_kernel_spmd(nc, in_maps, core_ids=list(range(NCORES)))
    LAST_RESULT = res

    # --- unshard: undo the sort permutation ---
    # device out layout: slot s -> out[s % 128, s // 128, :]
    full = np.empty((n_tok, D_PROJ), np.float32)
    for c in range(NCORES):
        oc = res.results[c]["out"]  # [128, T, 1024]
        oc_rows = oc.transpose(1, 0, 2).reshape(-1, D_PROJ)  # slot-major
        for (slot0, per_core_toks) in unshard:
            toks = per_core_toks[c]
            if len(toks):
                full[toks] = oc_rows[slot0 : slot0 + len(toks)]
    B, S = np.asarray(inp).shape
    return full.reshape(B, S, D_PROJ)


# revision 12
# speedup vs baseline: 1.1904x; 1.1904x over previous
"""Adaptive embedding (Transformer-XL wt103) on 8 trn2 NeuronCores.

Strategy: token-parallel across the 8 cores (2048 tokens each, no
collectives). Host sorts each core's tokens by id so that every
(bucket, 32k-row-block) becomes a contiguous segment; the device then
runs, per segment, a transposing dma_gather of the bf16 embedding rows
([d, tokens] layout, ready as the matmul stationary operand) followed by
PSUM-accumulated matmuls against the resident projection table, and
streams the projected rows out with contiguous DMAs. The host undoes the
sort permutation on the way back (unshard).

Tables are converted to bf16 host-side (rel err ~4e-3 against the f32
reference, well under the 2e-2 gate); projections are pre-transposed and
pre-scaled by sqrt(d_proj). The d=64/d=16 tables are zero-padded to 128
columns so every gather row is a multiple of 256 bytes and every matmul
runs with K=128.
"""

import os
import sys
import types

for _p in (
    "/root/.axon_site",
    "/root/.axon_site/_ro/trn_rl_repo",
    "/root/.axon_site/_ro/pypackages",
    "/opt/trn_rl_repo",
):
    if _p not in sys.path:
        sys.path.append(_p)

import numpy as np
import ml_dtypes

# antenv.axon_hooks shim: lets BASS_TRACE=1 profile runs work under axon.
try:
    import antenv.axon_hooks  # noqa: F401
except ImportError:
    _hooks = types.ModuleType("antenv.axon_hooks")
    _hooks._hook = None
    _hooks.set_axon_ntff_profile_hook = lambda h: setattr(_hooks, "_hook", h)
    _hooks.get_axon_ntff_profile_hook = lambda: _hooks._hook
    import antenv

    antenv.axon_hooks = _hooks
    sys.modules["antenv.axon_hooks"] = _hooks
    try:
        from trn_agent_boot.trn_boot import _ntff_profile_via_ctypes

        _h = _ntff_profile_via_ctypes("/opt/axon/libaxon_pjrt.so")
        if _h is not None:
            _hooks.set_axon_ntff_profile_hook(_h)
    except Exception:
        pass

import concourse.bacc as bacc
import concourse.mybir as mybir
import concourse.tile as tile
from concourse.bass_utils import run_bass_kernel_spmd

N_TOKEN = 267735
D_PROJ = 1024
CUTS = [0, 20000, 40000, 200000, N_TOKEN]
D_EMBS = [1024, 256, 64, 16]
D_PAD = [1024, 256, 128, 128]  # gathered row widths (>=128, %128)
EMB_SCALE = float(D_PROJ) ** 0.5
NCORES = 8
BLK = 32768  # int16 gather index range per segment
BF16 = ml_dtypes.bfloat16

# proj chunk bases within the packed [128, 12, 1024] projection tile
PROJ_CHUNK_BASE = [0, 8, 10, 11]

LAST_RESULT = None  # BassKernelResults of the most recent run (for test.py)


def _segments():
    """(bucket, local_row_base, id_lo, id_hi) for each gather segment."""
    segs = []
    for b in range(4):
        lo, hi = CUTS[b], CUTS[b + 1]
        for k in range(0, hi - lo, BLK):
            segs.append((b, k, lo + k, min(lo + k + BLK, hi)))
    return segs


def _build_graph(seg_plan, c_total, s_pad, rows):
    """seg_plan: list of (bucket, row_base, n_rows, n_pad, n_live, idx_colbase,
    slot_base), ordered smallest-gather-first (compute order)."""
    nc = bacc.Bacc(None, target_bir_lowering=False, debug=False, num_swdge_queues=4)
    dt = mybir.dt

    emb_par = [
        nc.declare_dram_parameter(f"embt{i}", [rows[i], D_PAD[i]], dt.bfloat16, False)
        for i in range(4)
    ]
    projs_par = nc.declare_dram_parameter("projs", [128, 12, 1024], dt.bfloat16, False)
    idx_par = nc.declare_dram_parameter("idxs", [128, c_total], dt.int16, False)
    # partition-major output: slot s lives at [s % 128, s // 128, :]
    out_par = nc.declare_dram_parameter(
        "out", [128, s_pad // 128, D_PROJ], dt.float32, True
    )

    # bucket order of first use in compute (for proj load ordering)
    border = []
    for (b, *_rest) in seg_plan:
        if b not in border:
            border.append(b)

    with tile.TileContext(nc) as tc:
        with (
            tc.tile_pool(name="const", bufs=1) as cpool,
            tc.tile_pool(name="et", bufs=1) as epool,
            tc.tile_pool(name="outs", bufs=4) as opool,
            tc.tile_pool(name="ps", bufs=8, space="PSUM") as ppool,
        ):
            # idx first, alone on the scalar HWDGE ring: the gathers (Pool)
            # wait only on this DMA, and the Pool engine must not run other
            # SWDGE DMAs (its dma_gather ucode library switch serializes
            # against in-flight Pool DMA work)
            idx_sb = cpool.tile([128, c_total], dt.int16, tag="idx")
            nc.scalar.dma_start(idx_sb[:], idx_par[:])

            # per-bucket projection tiles on the sync HWDGE ring, in compute
            # order; they drain while the gathers run
            proj_sb = [None] * 4
            for b in border:
                pcb, kc = PROJ_CHUNK_BASE[b], D_PAD[b] // 128
                pt = cpool.tile([128, kc, 1024], dt.bfloat16, tag=f"proj{b}")
                nc.sync.dma_start(pt[:], projs_par[:, pcb : pcb + kc, :])
                proj_sb[b] = pt

            # Issue all gathers up front; Tile pipelines them against compute.
            etiles = []
            for g, (b, row_base, n_rows, n_pad, n_live, cb, slot) in enumerate(
                seg_plan
            ):
                kc = D_PAD[b] // 128
                et = epool.tile([128, kc, n_pad], dt.bfloat16, tag=f"et{g}")
                nc.gpsimd.dma_gather(
                    out_ap=et[:],
                    in_ap=emb_par[b][row_base : row_base + n_rows, :],
                    idxs_ap=idx_sb[:, cb : cb + n_pad // 16],
                    num_idxs=n_pad,
                    num_idxs_reg=n_pad,
                    elem_size=D_PAD[b],
                    transpose=True,
                    queue_num=1 + g % 3,
                )
                etiles.append(et)

            n_out_tiles = 0
            n_out_dmas = 0
            for g, (b, row_base, n_rows, n_pad, n_live, cb, slot) in enumerate(
                seg_plan
            ):
                kc = D_PAD[b] // 128
                et = etiles[g]
                n_tiles = n_pad // 128
                # batch up to 4 token-tiles per output DMA (contiguous in the
                # partition-major out layout)
                for tb in range(0, n_tiles, 4):
                    gsz = min(4, n_tiles - tb)
                    out_sb = opool.tile(
                        [128, 4, D_PROJ], dt.float32, tag="osb", name="osb"
                    )
                    for ti in range(gsz):
                        tt = tb + ti
                        ps = [
                            ppool.tile([128, 512], dt.float32, tag="ps", name="ps0"),
                            ppool.tile([128, 512], dt.float32, tag="ps", name="ps1"),
                        ]
                        # K-chunk outer: both N-halves reuse the same stationary
                        for c in range(kc):
                            lhsT = et[:, c, tt * 128 : (tt + 1) * 128]
                            for nh in range(2):
                                nc.tensor.matmul(
                                    ps[nh][:],
                                    lhsT,
                                    proj_sb[b][:, c, nh * 512 : (nh + 1) * 512],
                                    start=(c == 0),
                                    stop=(c == kc - 1),
                                )
                        for nh in range(2):
                            dst = out_sb[:, ti, nh * 512 : (nh + 1) * 512]
                            if (n_out_tiles + nh) % 2 == 0:
                                nc.vector.tensor_copy(dst, ps[nh][:])
                            else:
                                nc.scalar.copy(dst, ps[nh][:])
                        n_out_tiles += 1
                    t0 = slot // 128 + tb
                    eng = nc.sync if n_out_dmas % 2 == 0 else nc.scalar
                    eng.dma_start(
                        out_par[:, t0 : t0 + gsz, :], out_sb[:, :gsz, :]
                    )
                    n_out_dmas += 1

    nc.compile()
    return nc


def kernel(inp, emb0, emb1, emb2, emb3, proj0, proj1, proj2, proj3):
    global LAST_RESULT
    ids = np.asarray(inp).reshape(-1).astype(np.int64)
    n_tok = ids.shape[0]
    assert n_tok % NCORES == 0

    embs = [np.asarray(e) for e in (emb0, emb1, emb2, emb3)]
    projs = [np.asarray(p) for p in (proj0, proj1, proj2, proj3)]

    # --- stage tables (bf16, small ones zero-padded to 128 cols) ---
    embs_b = []
    for b in range(4):
        e = embs[b].astype(BF16)
        if D_PAD[b] != D_EMBS[b]:
            e = np.concatenate(
                [e, np.zeros((e.shape[0], D_PAD[b] - D_EMBS[b]), BF16)], axis=1
            )
        embs_b.append(np.ascontiguousarray(e))
    rows = [e.shape[0] for e in embs_b]

    # packed projections: projT rows, scaled, padded, rearranged to [128,12,1024]
    pt = np.zeros((1536, D_PROJ), np.float32)
    r0 = 0
    for b in range(4):
        ptb = projs[b].T * EMB_SCALE  # [d_b, 1024]
        pt[r0 : r0 + D_EMBS[b]] = ptb
        r0 += D_PAD[b]
    projs_host = np.ascontiguousarray(
        pt.reshape(12, 128, D_PROJ).transpose(1, 0, 2).astype(BF16)
    )

    # --- sort + segment + deal to cores ---
    order = np.argsort(ids, kind="stable")
    sids = ids[order]

    # collect non-empty segments, then order smallest-gather-first so the
    # TensorEngine can start on quickly-landing data
    raw = []
    for b, row_base, id_lo, id_hi in _segments():
        g_lo = np.searchsorted(sids, id_lo, "left")
        g_hi = np.searchsorted(sids, id_hi, "left")
        if g_hi == g_lo:
            continue
        raw.append((b, row_base, id_lo, id_hi, g_lo, g_hi))
    raw.sort(key=lambda r: (-(-((r[5] - r[4] + NCORES - 1) // NCORES) // 128) * 128)
             * D_PAD[r[0]])

    seg_plan = []  # (bucket, row_base, n_rows, n_pad, n_live, idx_colbase, slot)
    core_idx = [[] for _ in range(NCORES)]  # per-core int16 idx arrays per seg
    unshard = []  # (slot_base, [global token positions per core])
    cb = 0
    slot = 0
    for b, row_base, id_lo, id_hi, g_lo, g_hi in raw:
        toks = order[g_lo:g_hi]
        locs = (sids[g_lo:g_hi] - id_lo).astype(np.int16)
        counts = [len(locs[c::NCORES]) for c in range(NCORES)]
        n_live = max(counts)
        n_pad = -(-n_live // 128) * 128
        per_core_toks = []
        for c in range(NCORES):
            li = locs[c::NCORES]
            pad = np.zeros(n_pad, np.int16)
            pad[: len(li)] = li
            core_idx[c].append(pad)
            per_core_toks.append(toks[c::NCORES])
        seg_plan.append((b, row_base, id_hi - id_lo, n_pad, n_live, cb, slot))
        unshard.append((slot, per_core_toks))
        cb += n_pad // 16
        slot += n_pad
    c_total = cb
    s_pad = slot

    # --- per-core wrapped idx tensors [128, c_total] int16 ---
    in_maps = []
    for c in range(NCORES):
        flat = np.concatenate(core_idx[c])
        w = flat.reshape(-1, 16).T  # [16, c_total]
        idx_host = np.ascontiguousarray(np.tile(w, (8, 1)))
        in_maps.append(
            {
                "embt0": embs_b[0],
                "embt1": embs_b[1],
                "embt2": embs_b[2],
                "embt3": embs_b[3],
                "projs": projs_host,
                "idxs": idx_host,
            }
        )

    nc = _build_graph(seg_plan, c_total, s_pad, rows)
    res = run_bass_kernel_spmd(nc, in_maps, core_ids=list(range(NCORES)))
    LAST_RESULT = res

    # --- unshard: undo the sort permutation ---
    # device out layout: slot s -> out[s % 128, s // 128, :]
    full = np.empty((n_tok, D_PROJ), np.float32)
    for c in range(NCORES):
        oc = res.results[c]["out"]  # [128, T, 1024]
        oc_rows = oc.transpose(1, 0, 2).reshape(-1, D_PROJ)  # slot-major
        for (slot0, per_core_toks) in unshard:
            toks = per_core_toks[c]
            if len(toks):
                full[toks] = oc_rows[slot0 : slot0 + len(toks)]
    B, S = np.asarray(inp).shape
    return full.reshape(B, S, D_PROJ)


# revision 13
# speedup vs baseline: 1.2101x; 1.0166x over previous
"""Adaptive embedding (Transformer-XL wt103) on 8 trn2 NeuronCores.

Strategy: token-parallel across the 8 cores (2048 tokens each, no
collectives). Host sorts each core's tokens by id so that every
(bucket, 32k-row-block) becomes a contiguous segment; the device then
runs, per segment, a transposing dma_gather of the bf16 embedding rows
([d, tokens] layout, ready as the matmul stationary operand) followed by
PSUM-accumulated matmuls against the resident projection table, and
streams the projected rows out with contiguous DMAs. The host undoes the
sort permutation on the way back (unshard).

Tables are converted to bf16 host-side (rel err ~4e-3 against the f32
reference, well under the 2e-2 gate); projections are pre-transposed and
pre-scaled by sqrt(d_proj). The d=64/d=16 tables are zero-padded to 128
columns so every gather row is a multiple of 256 bytes and every matmul
runs with K=128.
"""

import os
import sys
import types

for _p in (
    "/root/.axon_site",
    "/root/.axon_site/_ro/trn_rl_repo",
    "/root/.axon_site/_ro/pypackages",
    "/opt/trn_rl_repo",
):
    if _p not in sys.path:
        sys.path.append(_p)

import numpy as np
import ml_dtypes

# antenv.axon_hooks shim: lets BASS_TRACE=1 profile runs work under axon.
try:
    import antenv.axon_hooks  # noqa: F401
except ImportError:
    _hooks = types.ModuleType("antenv.axon_hooks")
    _hooks._hook = None
    _hooks.set_axon_ntff_profile_hook = lambda h: setattr(_hooks, "_hook", h)
    _hooks.get_axon_ntff_profile_hook = lambda: _hooks._hook
    import antenv

    antenv.axon_hooks = _hooks
    sys.modules["antenv.axon_hooks"] = _hooks
    try:
        from trn_agent_boot.trn_boot import _ntff_profile_via_ctypes

        _h = _ntff_profile_via_ctypes("/opt/axon/libaxon_pjrt.so")
        if _h is not None:
            _hooks.set_axon_ntff_profile_hook(_h)
    except Exception:
        pass

import concourse.bacc as bacc
import concourse.mybir as mybir
import concourse.tile as tile
from concourse.bass_utils import run_bass_kernel_spmd

N_TOKEN = 267735
D_PROJ = 1024
CUTS = [0, 20000, 40000, 200000, N_TOKEN]
D_EMBS = [1024, 256, 64, 16]
D_PAD = [1024, 256, 128, 128]  # gathered row widths (>=128, %128)
EMB_SCALE = float(D_PROJ) ** 0.5
NCORES = 8
BLK = 32768  # int16 gather index range per segment
BF16 = ml_dtypes.bfloat16

# proj chunk bases within the packed [128, 12, 1024] projection tile
PROJ_CHUNK_BASE = [0, 8, 10, 11]

LAST_RESULT = None  # BassKernelResults of the most recent run (for test.py)


def _segments():
    """(bucket, local_row_base, id_lo, id_hi) for each gather segment."""
    segs = []
    for b in range(4):
        lo, hi = CUTS[b], CUTS[b + 1]
        for k in range(0, hi - lo, BLK):
            segs.append((b, k, lo + k, min(lo + k + BLK, hi)))
    return segs


def _build_graph(seg_plan, c_total, s_pad, rows):
    """seg_plan: list of (bucket, row_base, n_rows, n_pad, n_live, idx_colbase,
    slot_base), ordered smallest-gather-first (compute order)."""
    nc = bacc.Bacc(None, target_bir_lowering=False, debug=False, num_swdge_queues=4)
    dt = mybir.dt

    emb_par = [
        nc.declare_dram_parameter(f"embt{i}", [rows[i], D_PAD[i]], dt.bfloat16, False)
        for i in range(4)
    ]
    projs_par = nc.declare_dram_parameter("projs", [128, 12, 1024], dt.bfloat16, False)
    idx_par = nc.declare_dram_parameter("idxs", [128, c_total], dt.int16, False)
    # partition-major output: slot s lives at [s % 128, s // 128, :]
    out_par = nc.declare_dram_parameter(
        "out", [128, s_pad // 128, D_PROJ], dt.float32, True
    )

    # bucket order of first use in compute (for proj load ordering)
    border = []
    for (b, *_rest) in seg_plan:
        if b not in border:
            border.append(b)

    with tile.TileContext(nc) as tc:
        with (
            tc.tile_pool(name="const", bufs=1) as cpool,
            tc.tile_pool(name="et", bufs=1) as epool,
            tc.tile_pool(name="outs", bufs=4) as opool,
            tc.tile_pool(name="ps", bufs=8, space="PSUM") as ppool,
        ):
            # idx first, alone on the scalar HWDGE ring: the gathers (Pool)
            # wait only on this DMA, and the Pool engine must not run other
            # SWDGE DMAs (its dma_gather ucode library switch serializes
            # against in-flight Pool DMA work)
            idx_sb = cpool.tile([128, c_total], dt.int16, tag="idx")
            nc.sync.dma_start(idx_sb[:], idx_par[:])

            # per-bucket projection tiles on the sync HWDGE ring, in compute
            # order; they drain while the gathers run
            proj_sb = [None] * 4
            for b in border:
                pcb, kc = PROJ_CHUNK_BASE[b], D_PAD[b] // 128
                pt = cpool.tile([128, kc, 1024], dt.bfloat16, tag=f"proj{b}")
                nc.sync.dma_start(pt[:], projs_par[:, pcb : pcb + kc, :])
                proj_sb[b] = pt

            # Issue all gathers up front; Tile pipelines them against compute.
            etiles = []
            for g, (b, row_base, n_rows, n_pad, n_live, cb, slot) in enumerate(
                seg_plan
            ):
                kc = D_PAD[b] // 128
                et = epool.tile([128, kc, n_pad], dt.bfloat16, tag=f"et{g}")
                nc.gpsimd.dma_gather(
                    out_ap=et[:],
                    in_ap=emb_par[b][row_base : row_base + n_rows, :],
                    idxs_ap=idx_sb[:, cb : cb + n_pad // 16],
                    num_idxs=n_pad,
                    num_idxs_reg=n_pad,
                    elem_size=D_PAD[b],
                    transpose=True,
                    queue_num=1 + g % 3,
                )
                etiles.append(et)

            n_out_tiles = 0
            n_out_dmas = 0
            for g, (b, row_base, n_rows, n_pad, n_live, cb, slot) in enumerate(
                seg_plan
            ):
                kc = D_PAD[b] // 128
                et = etiles[g]
                n_tiles = n_pad // 128
                # batch up to 4 token-tiles per output DMA (contiguous in the
                # partition-major out layout)
                for tb in range(0, n_tiles, 4):
                    gsz = min(4, n_tiles - tb)
                    out_sb = opool.tile(
                        [128, 4, D_PROJ], dt.float32, tag="osb", name="osb"
                    )
                    for ti in range(gsz):
                        tt = tb + ti
                        ps = [
                            ppool.tile([128, 512], dt.float32, tag="ps", name="ps0"),
                            ppool.tile([128, 512], dt.float32, tag="ps", name="ps1"),
                        ]
                        # K-chunk outer: both N-halves reuse the same stationary
                        for c in range(kc):
                            lhsT = et[:, c, tt * 128 : (tt + 1) * 128]
                            for nh in range(2):
                                nc.tensor.matmul(
                                    ps[nh][:],
                                    lhsT,
                                    proj_sb[b][:, c, nh * 512 : (nh + 1) * 512],
                                    start=(c == 0),
                                    stop=(c == kc - 1),
                                )
                        for nh in range(2):
                            dst = out_sb[:, ti, nh * 512 : (nh + 1) * 512]
                            if (n_out_tiles + nh) % 2 == 0:
                                nc.vector.tensor_copy(dst, ps[nh][:])
                            else:
                                nc.scalar.copy(dst, ps[nh][:])
                        n_out_tiles += 1
                    t0 = slot // 128 + tb
                    eng = nc.sync if n_out_dmas % 2 == 0 else nc.scalar
                    eng.dma_start(
                        out_par[:, t0 : t0 + gsz, :], out_sb[:, :gsz, :]
                    )
                    n_out_dmas += 1

    nc.compile()
    return nc


def kernel(inp, emb0, emb1, emb2, emb3, proj0, proj1, proj2, proj3):
    global LAST_RESULT
    ids = np.asarray(inp).reshape(-1).astype(np.int64)
    n_tok = ids.shape[0]
    assert n_tok % NCORES == 0

    embs = [np.asarray(e) for e in (emb0, emb1, emb2, emb3)]
    projs = [np.asarray(p) for p in (proj0, proj1, proj2, proj3)]

    # --- stage tables (bf16, small ones zero-padded to 128 cols) ---
    embs_b = []
    for b in range(4):
        e = embs[b].astype(BF16)
        if D_PAD[b] != D_EMBS[b]:
            e = np.concatenate(
                [e, np.zeros((e.shape[0], D_PAD[b] - D_EMBS[b]), BF16)], axis=1
            )
        embs_b.append(np.ascontiguousarray(e))
    rows = [e.shape[0] for e in embs_b]

    # packed projections: projT rows, scaled, padded, rearranged to [128,12,1024]
    pt = np.zeros((1536, D_PROJ), np.float32)
    r0 = 0
    for b in range(4):
        ptb = projs[b].T * EMB_SCALE  # [d_b, 1024]
        pt[r0 : r0 + D_EMBS[b]] = ptb
        r0 += D_PAD[b]
    projs_host = np.ascontiguousarray(
        pt.reshape(12, 128, D_PROJ).transpose(1, 0, 2).astype(BF16)
    )

    # --- sort + segment + deal to cores ---
    order = np.argsort(ids, kind="stable")
    sids = ids[order]

    # collect non-empty segments, then order smallest-gather-first so the
    # TensorEngine can start on quickly-landing data
    raw = []
    for b, row_base, id_lo, id_hi in _segments():
        g_lo = np.searchsorted(sids, id_lo, "left")
        g_hi = np.searchsorted(sids, id_hi, "left")
        if g_hi == g_lo:
            continue
        raw.append((b, row_base, id_lo, id_hi, g_lo, g_hi))
    raw.sort(key=lambda r: (-(-((r[5] - r[4] + NCORES - 1) // NCORES) // 128) * 128)
             * D_PAD[r[0]])

    seg_plan = []  # (bucket, row_base, n_rows, n_pad, n_live, idx_colbase, slot)
    core_idx = [[] for _ in range(NCORES)]  # per-core int16 idx arrays per seg
    unshard = []  # (slot_base, [global token positions per core])
    cb = 0
    slot = 0
    for b, row_base, id_lo, id_hi, g_lo, g_hi in raw:
        toks = order[g_lo:g_hi]
        locs = (sids[g_lo:g_hi] - id_lo).astype(np.int16)
        counts = [len(locs[c::NCORES]) for c in range(NCORES)]
        n_live = max(counts)
        n_pad = -(-n_live // 128) * 128
        per_core_toks = []
        for c in range(NCORES):
            li = locs[c::NCORES]
            pad = np.zeros(n_pad, np.int16)
            pad[: len(li)] = li
            core_idx[c].append(pad)
            per_core_toks.append(toks[c::NCORES])
        seg_plan.append((b, row_base, id_hi - id_lo, n_pad, n_live, cb, slot))
        unshard.append((slot, per_core_toks))
        cb += n_pad // 16
        slot += n_pad
    c_total = cb
    s_pad = slot

    # --- per-core wrapped idx tensors [128, c_total] int16 ---
    in_maps = []
    for c in range(NCORES):
        flat = np.concatenate(core_idx[c])
        w = flat.reshape(-1, 16).T  # [16, c_total]
        idx_host = np.ascontiguousarray(np.tile(w, (8, 1)))
        in_maps.append(
            {
                "embt0": embs_b[0],
                "embt1": embs_b[1],
                "embt2": embs_b[2],
                "embt3": embs_b[3],
                "projs": projs_host,
                "idxs": idx_host,
            }
        )

    nc = _build_graph(seg_plan, c_total, s_pad, rows)
    res = run_bass_kernel_spmd(nc, in_maps, core_ids=list(range(NCORES)))
    LAST_RESULT = res

    # --- unshard: undo the sort permutation ---
    # device out layout: slot s -> out[s % 128, s // 128, :]
    full = np.empty((n_tok, D_PROJ), np.float32)
    for c in range(NCORES):
        oc = res.results[c]["out"]  # [128, T, 1024]
        oc_rows = oc.transpose(1, 0, 2).reshape(-1, D_PROJ)  # slot-major
        for (slot0, per_core_toks) in unshard:
            toks = per_core_toks[c]
            if len(toks):
                full[toks] = oc_rows[slot0 : slot0 + len(toks)]
    B, S = np.asarray(inp).shape
    return full.reshape(B, S, D_PROJ)
